# revision 1
# baseline (speedup 1.0000x reference)
"""Trainium2 Bass kernel for a 6-layer 4D CNN (3^4 SAME convs + PReLU).

Problem: x (8, 2, 16,16,16,16) -> 6 conv layers, channels 2->32->32->32->32
->32->2, PReLU (scalar slope) after the first five convs.

Strategy (per NeuronCore, data-parallel over batch N=8 across 8 cores):
  * d-axis banded-Toeplitz matmuls: activations live in SBUF in a
    "decimated" layout X'[32*s + ch, (a, b, c_pad, j)] where block s in 0..3
    holds d = 2*j + s - 1 (d-phases).  One matmul then contracts
    K = 128 = (4 d-phases x 32 ch) and produces M = 64 = (2 d-outs x 32 ch)
    outputs per column: the 3-tap d-convolution is folded into the
    stationary (block-banded) weight matrix.
  * (da, db, dc) taps: 27 PSUM-accumulated matmuls with shifted windows;
    c is zero-padded in the layout (no clipping), b clipped via windows,
    a via skip.
  * 2-way column packing (tile_position (0,0) / (0,64)) runs two spatial
    chunks concurrently on the 128x128 PE array.
  * Layer 0 (2 input channels) uses an a-partition scheme: partitions =
    (ch, a), M = 128 = (4 a-outs x 32 ch), a-banded stationaries; the dd
    taps are folded into K = 96 via three d-shifted input replicas, so the
    27 taps collapse to 9 (db, dc) matmul groups.  (Concurrent row-group
    tile_position matmuls accumulating into one PSUM bank fault on HW —
    single full-K matmuls are used throughout.)
  * Layer 5 (2 output channels) uses M = 4 = (2 ch x 2 d-outs) with 4-way
    column packing; result leaves in decimated layout, host reassembles.
  * PReLU(x) = max(x, slope*x) via one scalar_tensor_tensor op, fused with
    the psum->SBUF eviction; fp16 activations, fp32 PSUM accumulation.
"""

import sys

import numpy as np

for _p in ("/opt/trn_rl_repo", "/root/.axon_site/_ro/trn_rl_repo"):
    if _p not in sys.path:
        sys.path.append(_p)

import concourse.bass as bass  # noqa: E402
import concourse.mybir as mybir  # noqa: E402
import concourse.tile as tile  # noqa: E402
from concourse import bacc, bass_utils  # noqa: E402
from concourse._compat import with_exitstack  # noqa: E402

F32 = mybir.dt.float32
F16 = mybir.dt.float16

LB = 16
CP = 18   # padded c axis (c in -1..16)
DP = 18   # padded d axis in x_pad
J = 8     # d//2

# tap orderings (db major with db=0 first so the first matmul of every psum
# chunk covers the full window and can carry start=True)
G_MID = [(da, db, dc)
         for db in (0, -1, 1) for da in (0, -1, 1) for dc in (0, -1, 1)]
# L0 groups: (db, dc) only — the dd taps are folded into K=96 via three
# d-shifted partition-block replicas of the input (block rg holds x shifted
# by dd = rg - 1)
G_L0 = [(db, dc) for db in (0, -1, 1) for dc in (0, -1, 1)]


def _pack_weights(ks, la):
    """Host-side packing of conv kernels into stationary matrices (fp16)."""
    na4 = la // 4
    k0, k1, k2, k3, k4, k5 = [np.asarray(k, np.float32) for k in ks]

    # L0: W0[32*rg + i*la + a_in, (g*na4 + a0b)*128 + a_j*32 + o]
    # K = 96 = 3 d-shift blocks (rg -> dd = rg - 1) x (2 ch x la a_in, banded)
    w0 = np.zeros((128, len(G_L0) * na4 * 128), np.float32)
    for gi, (db, dc) in enumerate(G_L0):
        for a0b in range(na4):
            cb = (gi * na4 + a0b) * 128
            for rg in range(3):
                for aj in range(4):
                    for da in (-1, 0, 1):
                        ain = a0b * 4 + aj + da
                        if not (0 <= ain < la):
                            continue
                        for i in range(2):
                            w0[32 * rg + i * la + ain,
                               cb + aj * 32:cb + aj * 32 + 32] = \
                                k0[:, i, da + 1, db + 1, dc + 1, rg]

    # mid layers: W[32*s + i, g*64 + t*32 + o]
    def pack_mid(k):
        w = np.zeros((128, 27 * 64), np.float32)
        for gi, (da, db, dc) in enumerate(G_MID):
            for s in range(4):
                for t in range(2):
                    if 0 <= s - t <= 2:
                        w[32 * s:32 * s + 32, gi * 64 + t * 32:gi * 64 + t * 32 + 32] = \
                            k[:, :, da + 1, db + 1, dc + 1, s - t].T
        return w

    # L5: W5[32*s + i, g*4 + o*2 + t]
    w5 = np.zeros((128, 27 * 4), np.float32)
    for gi, (da, db, dc) in enumerate(G_MID):
        for s in range(4):
            for t in range(2):
                if 0 <= s - t <= 2:
                    for o in range(2):
                        w5[32 * s:32 * s + 32, gi * 4 + o * 2 + t] = \
                            k5[o, :, da + 1, db + 1, dc + 1, s - t]

    return ([w0.astype(np.float16)] +
            [pack_mid(k).astype(np.float16) for k in (k1, k2, k3, k4)] +
            [w5.astype(np.float16)])


@with_exitstack
def _conv_kernel(ctx, tc, la, slopes):
    """Emit the full 6-layer conv program. slopes: python floats len 5."""
    nc = tc.nc
    na4 = la // 4
    xcols = la * LB * CP * J
    pcols = LB * CP * DP

    xin = nc.dram_tensor("xin", [2 * la, 4096], F32, kind="ExternalInput")
    w0_d = nc.dram_tensor("w0", [128, len(G_L0) * na4 * 128],
                          F16, kind="ExternalInput")
    wmid_d = [nc.dram_tensor(f"w{l}", [128, 27 * 64], F16,
                             kind="ExternalInput") for l in (1, 2, 3, 4)]
    w5_d = nc.dram_tensor("w5", [128, 27 * 4], F16, kind="ExternalInput")
    out_d = nc.dram_tensor("out_dec", [4, la * 2048], F32,
                           kind="ExternalOutput")

    const = ctx.enter_context(tc.tile_pool(name="const", bufs=1))
    pp = ctx.enter_context(tc.tile_pool(name="ps", bufs=8, space="PSUM"))
    stg = ctx.enter_context(tc.tile_pool(name="stg", bufs=4))

    # ---- load weights ----
    w0t = const.tile([128, w0_d.shape[1]], F16)
    nc.sync.dma_start(w0t[:], w0_d[:])
    wmt = []
    for wd in wmid_d:
        t = const.tile([128, 27 * 64], F16, tag=wd.name)
        nc.sync.dma_start(t[:], wd[:])
        wmt.append(t)
    w5t = const.tile([128, 27 * 4], F16)
    nc.sync.dma_start(w5t[:], w5_d[:])

    # ---- build x_pad [128, (b, c_pad, d_pad)] fp16, replicated x4 ----
    xsb = const.tile([2 * la, 4096], F32)
    nc.sync.dma_start(xsb[:], xin[:])
    xpad = const.tile([128, pcols], F16)
    nc.vector.memset(xpad[:], 0.0)
    xp4 = xpad.rearrange("p (b c d) -> p b c d", b=LB, c=CP, d=DP)
    xs4 = xsb.rearrange("p (b c d) -> p b c d", b=LB, c=16, d=16)
    # block rg holds x shifted in d by dd = rg - 1 (zero-padded edges)
    for rg in range(3):
        dst = xp4[32 * rg:32 * rg + 2 * la, :, 1:17, 2 - rg:18 - rg]
        if rg == 1:
            nc.vector.tensor_copy(dst, xs4[:])
        else:
            nc.scalar.copy(dst, xs4[:])

    # ---- X' ping-pong buffers ----
    xa = const.tile([128, xcols], F16)
    xb = const.tile([128, xcols], F16)
    nc.gpsimd.memset(xa[:], 0.0)
    nc.gpsimd.memset(xb[:], 0.0)
    bufs = [xa, xb]

    def xview(t):
        return t.rearrange("p (a b c j) -> p a b c j", a=la, b=LB, c=CP, j=J)

    # scatter map: s -> (j_lo, j_cnt, d_lo)  [d = 2j + s - 1]
    SMAP = {0: (1, 7, 1), 1: (0, 8, 0), 2: (0, 8, 1), 3: (0, 7, 2)}

    # ================= layer 0 =================
    xn4 = xview(bufs[0])
    a_slope = slopes[0]
    for a0b in range(na4):
        for bc in range(8):          # b-pair chunks
            b0 = bc * 2
            ps = pp.tile([128, 512], F32, tag="ps")
            p4 = ps.rearrange("p (b c d) -> p b c d", b=2, c=16, d=16)
            for gi, (db, dc) in enumerate(G_L0):
                blo = max(b0, -db)
                bhi = min(b0 + 2, 16 - db)
                cb = (gi * na4 + a0b) * 128
                # K = 96: three d-shift blocks; rhs d-slice 1:17 uniform
                rhs = xp4[0:96, blo + db:bhi + db, dc + 1:dc + 17, 1:17]
                out = p4[:, blo - b0:bhi - b0, :, :]
                nc.tensor.matmul(out, w0t[0:96, cb:cb + 128], rhs,
                                 start=(gi == 0),
                                 stop=(gi == len(G_L0) - 1))
            # prelu the whole chunk into fp16 staging, then scatter
            sg = stg.tile([128, 512], F16, tag="l0st")
            nc.scalar.activation(sg[:], ps[:],
                                 mybir.ActivationFunctionType.Relu,
                                 scale=1.0 - a_slope)
            nc.vector.scalar_tensor_tensor(
                sg[:], ps[:], a_slope, sg[:],
                op0=mybir.AluOpType.mult, op1=mybir.AluOpType.add)
            sg4 = sg.rearrange("p (b c d) -> p b c d", b=2, c=16, d=16)
            for aj in range(4):
                a = a0b * 4 + aj
                for s in range(4):
                    jlo, jcnt, dlo = SMAP[s]
                    dst = xn4[32 * s:32 * s + 32, a, b0:b0 + 2, 1:17,
                              jlo:jlo + jcnt]
                    src = sg4[32 * aj:32 * aj + 32, :, :,
                              dlo:dlo + 2 * jcnt - 1:2]
                    if s in (0, 2):
                        nc.scalar.copy(dst, src)
                    else:
                        nc.vector.tensor_copy(dst, src)

    # ================= layers 1..4 =================
    for l in range(1, 5):
        xc4 = xview(bufs[(l + 1) % 2])
        xn4 = xview(bufs[l % 2])
        wt = wmt[l - 1]
        a_slope = slopes[l]
        for a in range(la):
            for half in range(2):
                # one PSUM bank per col-packed chunk; interleave the two
                # chunks' matmuls so their PE streams overlap (different
                # column groups of the array)
                pst = [pp.tile([128, 512], F32, tag="ps", name=f"psm{ci}") for ci in range(2)]
                mms = []
                for gi, (da, db, dc) in enumerate(G_MID):
                    if not (0 <= a + da < la):
                        continue
                    mms.append((gi, da, db, dc))
                nmm = len(mms)
                prev_mm = None
                for idx, (gi, da, db, dc) in enumerate(mms):
                    for ci in range(2):
                        b0 = half * 8 + ci * 4
                        rb = ci * 64
                        blo = max(b0, -db)
                        bhi = min(b0 + 4, 16 - db)
                        rhs = xc4[:, a + da, blo + db:bhi + db,
                                  dc + 1:dc + 17, :]
                        out = pst[ci][rb:rb + 64,
                                      (blo - b0) * 128:(bhi - b0) * 128]
                        mm = nc.tensor.matmul(
                            out, wt[:, gi * 64:gi * 64 + 64], rhs,
                            start=(idx == 0), stop=(idx == nmm - 1),
                            tile_position=(0, rb))
                        # keep A/B col-group streams interleaved on the PE
                        # queue so they overlap on distinct array columns
                        if prev_mm is not None:
                            tile.add_dep_helper(mm.ins, prev_mm.ins, sync=False,
                                                reason="colpack order")
                        prev_mm = mm
                for ci in range(2):
                    b0 = half * 8 + ci * 4
                    rb = ci * 64
                    # t=0 -> s'=1 direct ; t=1 -> s'=2 direct
                    # prelu(x) = (1-a)*relu(x) + a*x in two single-psum ops
                    for t, sp in ((0, 1), (1, 2)):
                        dst = xn4[32 * sp:32 * sp + 32, a, b0:b0 + 4,
                                  1:17, 0:8]
                        src = pst[ci][rb + 32 * t:rb + 32 * t + 32, :]
                        nc.scalar.activation(
                            dst, src, mybir.ActivationFunctionType.Relu,
                            scale=1.0 - a_slope)
                        nc.vector.scalar_tensor_tensor(
                            dst, src, a_slope, dst,
                            op0=mybir.AluOpType.mult,
                            op1=mybir.AluOpType.add)
                    # j-shift copies: s'=3 <- s'=1 (j+1) ; s'=0 <- s'=2 (j-1)
                    # split across ACT/DVE so neither eviction engine stalls
                    # the PE stream
                    nc.scalar.copy(
                        xn4[96:128, a, b0:b0 + 4, 1:17, 0:7],
                        xn4[32:64, a, b0:b0 + 4, 1:17, 1:8])
                    nc.vector.tensor_copy(
                        xn4[0:32, a, b0:b0 + 4, 1:17, 1:8],
                        xn4[64:96, a, b0:b0 + 4, 1:17, 0:7])

    # ================= layer 5 =================
    xc4 = xview(bufs[0])
    for a in range(la):
        pst = [pp.tile([128, 512], F32, tag="ps", name=f"ps5{q}") for q in range(4)]
        mms = []
        for gi, (da, db, dc) in enumerate(G_MID):
            if not (0 <= a + da < la):
                continue
            mms.append((gi, da, db, dc))
        nmm = len(mms)
        prev_mm = None
        for idx, (gi, da, db, dc) in enumerate(mms):
            for q in range(4):
                b0 = q * 4
                blo = max(b0, -db)
                bhi = min(b0 + 4, 16 - db)
                rhs = xc4[:, a + da, blo + db:bhi + db, dc + 1:dc + 17, :]
                out = pst[q][32 * q:32 * q + 4,
                             (blo - b0) * 128:(bhi - b0) * 128]
                mm = nc.tensor.matmul(out, w5t[:, gi * 4:gi * 4 + 4], rhs,
                                      start=(idx == 0), stop=(idx == nmm - 1),
                                      tile_position=(0, 32 * q))
                if prev_mm is not None:
                    tile.add_dep_helper(mm.ins, prev_mm.ins, sync=False,
                                        reason="colpack order")
                prev_mm = mm
        for q in range(4):
            st = stg.tile([4, 512], F32, tag="stg")
            nc.vector.tensor_copy(st[:], pst[q][32 * q:32 * q + 4, :])
            cb = a * 2048 + q * 512
            nc.sync.dma_start(out_d[:, cb:cb + 512], st[:])


_CACHE = {}
LAST_RESULT = None


def _build(la, slopes):
    key = (la, tuple(slopes))
    if key not in _CACHE:
        nc = bacc.Bacc("TRN2")
        with tile.TileContext(nc) as tc:
            _conv_kernel(tc, la, slopes)
        nc.compile()
        _CACHE[key] = nc
    return _CACHE[key]


def kernel(x, k0, k1, k2, k3, k4, k5, slopes):
    x = np.asarray(x, np.float32)
    n, _, la = x.shape[:3]
    slopes_f = [float(s) for s in np.asarray(slopes, np.float32)]
    ws = _pack_weights((k0, k1, k2, k3, k4, k5), la)
    nc = _build(la, slopes_f)

    in_maps = []
    for i in range(n):
        m = {"xin": np.ascontiguousarray(x[i].reshape(2 * la, 4096)),
             "w0": ws[0], "w5": ws[5]}
        for l in (1, 2, 3, 4):
            m[f"w{l}"] = ws[l]
        in_maps.append(m)

    res = bass_utils.run_bass_kernel_spmd(nc, in_maps,
                                          core_ids=list(range(n)))
    global LAST_RESULT
    LAST_RESULT = res
    outs = []
    for i in range(n):
        od = res.results[i]["out_dec"].reshape(2, 2, la, 16, 16, 8)
        # [o, t, a, b, c, j] -> [o, a, b, c, j, t] -> d = 2j + t
        o = np.transpose(od, (0, 2, 3, 4, 5, 1)).reshape(2, la, 16, 16, 16)
        outs.append(o)
    return np.stack(outs).astype(np.float32)



# revision 26
# speedup vs baseline: 1.5094x; 1.5094x over previous
"""Trainium2 Bass kernel for a 6-layer 4D CNN (3^4 SAME convs + PReLU).

Problem: x (8, 2, 16,16,16,16) -> 6 conv layers, channels 2->32->32->32->32
->32->2, PReLU (scalar slope) after the first five convs.

Strategy (per NeuronCore, data-parallel over batch N=8 across 8 cores):
  * d-axis banded-Toeplitz matmuls: activations live in SBUF in a
    "decimated" layout X''[32*s + ch, (a, b, c, j)] where block s in 0..3
    holds d = 2*j + s - 1 (d-phases).  One matmul contracts
    K = 128 = (4 d-phases x 32 ch) and produces M = 64 = (2 d-outs x 32 ch)
    outputs per column: the 3-tap d-convolution is folded into the
    stationary (block-banded) weight matrix.
  * Winograd F(2,3)^2 over the (b, c) axes for layers 1..5: the 9 (db, dc)
    taps become 16 independent transformed points (xi_b, xi_c in 0..3) with
    only the 3 da taps left as PSUM accumulation.  Streamed matmul columns
    per layer drop from 27 * (out/64) to 16 * 3 * (out/128): 2.25x less
    TensorE time.  Forward/inverse transforms are +-1 adds done on the
    Vector/GpSimd engines; PReLU (ACT Lrelu) applies in the spatial domain
    between inverse and the next forward transform.
  * Layer 0 (2 input channels) uses an a-partition scheme: partitions =
    (ch, a), M = 128 = (4 a-outs x 32 ch), a-banded stationaries; the dd
    taps are folded into K = 96 via three d-shifted input replicas, so the
    27 taps collapse to 9 (db, dc) matmul groups.
  * Layer 5 (2 output channels) reuses the Winograd path with M = 4
    (2 ch x 2 d-outs) packed 4-per-PSUM-bank; result leaves in decimated
    (o, t) layout, host reassembles d = 2j + t.
  * fp16 activations and weights, fp32 PSUM accumulation.
"""

import sys

import numpy as np

for _p in ("/opt/trn_rl_repo", "/root/.axon_site/_ro/trn_rl_repo"):
    if _p not in sys.path:
        sys.path.append(_p)

import concourse.bass as bass  # noqa: E402
import concourse.mybir as mybir  # noqa: E402
import concourse.tile as tile  # noqa: E402
from concourse import bacc, bass_utils  # noqa: E402
from concourse._compat import with_exitstack  # noqa: E402

F32 = mybir.dt.float32
F16 = mybir.dt.float16
ADD = mybir.AluOpType.add
SUB = mybir.AluOpType.subtract

LB = 16
CP = 18   # padded c axis for the L0 input replica
DP = 18   # padded d axis in x_pad
J = 8     # d//2

USE_LRELU = True

# L0 groups: (db, dc) only — the dd taps are folded into K=96 via three
# d-shifted partition-block replicas of the input
G_L0 = [(db, dc) for db in (0, -1, 1) for dc in (0, -1, 1)]

# physical partition offset of logical d-phase block s in X''/X-tilde:
# direct PReLU outputs (s=1,2) sit at base 0 so the [64]-partition ACT
# write is 64-aligned (HW: >32-partition access must be 64-aligned)
PHYS = {0: 64, 1: 0, 2: 32, 3: 96}

# Winograd F(2,3) matrices (cross-correlation form)
_G = np.array([[1, 0, 0], [.5, .5, .5], [.5, -.5, .5], [0, 0, 1]], np.float32)


def _pack_weights(ks, la):
    """Host-side packing of conv kernels into stationary matrices (fp16)."""
    na4 = la // 4
    k0, k1, k2, k3, k4, k5 = [np.asarray(k, np.float32) for k in ks]

    # L0: W0[32*rg + i*la + a_in, (g*na4 + a0b)*128 + a_j*32 + o]
    w0 = np.zeros((128, len(G_L0) * na4 * 128), np.float32)
    for gi, (db, dc) in enumerate(G_L0):
        for a0b in range(na4):
            cb = (gi * na4 + a0b) * 128
            for rg in range(3):
                for aj in range(4):
                    for da in (-1, 0, 1):
                        ain = a0b * 4 + aj + da
                        if not (0 <= ain < la):
                            continue
                        for i in range(2):
                            w0[32 * rg + i * la + ain,
                               cb + aj * 32:cb + aj * 32 + 32] = \
                                k0[:, i, da + 1, db + 1, dc + 1, rg]

    # winograd-transformed mid layers:
    # khat[o,i,da,xb,xc,dd] = sum_{db,dc} G[xb,db] G[xc,dc] k[o,i,da,db,dc,dd]
    # W[32s+i, ((xb*4+xc)*3 + dai)*64 + t*32 + o] = khat[o,i,dai,xb,xc,s-t]
    def pack_mid_wino(k):
        kh = np.einsum("up,vq,oiapqd->oiauvd", _G, _G, k, optimize=True)
        w = np.zeros((128, 16 * 3 * 64), np.float32)
        for xb in range(4):
            for xc in range(4):
                for dai in range(3):
                    cb = (((xb * 4 + xc) * 3) + dai) * 64
                    for s in range(4):
                        for t in range(2):
                            if 0 <= s - t <= 2:
                                w[PHYS[s]:PHYS[s] + 32,
                                  cb + t * 32:cb + t * 32 + 32] = \
                                    kh[:, :, dai, xb, xc, s - t].T
        return w

    # L5 winograd: W5[32s+i, ((xb*4+xc)*3 + dai)*4 + o*2 + t]
    kh5 = np.einsum("up,vq,oiapqd->oiauvd", _G, _G, k5, optimize=True)
    w5 = np.zeros((128, 16 * 3 * 4), np.float32)
    for xb in range(4):
        for xc in range(4):
            for dai in range(3):
                cb = (((xb * 4 + xc) * 3) + dai) * 4
                for s in range(4):
                    for t in range(2):
                        if 0 <= s - t <= 2:
                            for o in range(2):
                                w5[PHYS[s]:PHYS[s] + 32, cb + o * 2 + t] = \
                                    kh5[o, :, dai, xb, xc, s - t]

    return ([w0.astype(np.float16)] +
            [pack_mid_wino(k).astype(np.float16) for k in (k1, k2, k3, k4)] +
            [w5.astype(np.float16)])


def _fwd_transform(nc, xw4, tbf, xtv, slot, a):
    """Forward Winograd transform of X'' slice a -> ring slot.

    xw4: X'' view [128, a, b16, c16, j8]; tbf: [128, 4096] staging;
    xtv: ring view [128, slot, xb, xc, bt, ct, j]."""
    t4 = tbf.rearrange("p (xb bt c j) -> p xb bt c j", xb=4, bt=8, c=16, j=8)
    x3 = xw4[:, a]
    # b-stage: B^T rows over b-windows 2bt-1..2bt+2
    # r0 = x[2bt-1] - x[2bt+1]   (bt=0 edge: -x[1])
    nc.vector.tensor_tensor(t4[:, 0, 1:8], x3[:, 1:14:2], x3[:, 3:16:2], op=SUB)
    nc.vector.tensor_scalar_mul(t4[:, 0, 0:1], x3[:, 1:2], -1.0)
    # r1 = x[2bt] + x[2bt+1]
    nc.vector.tensor_tensor(t4[:, 1], x3[:, 0:16:2], x3[:, 1:16:2], op=ADD)
    # r2 = x[2bt+1] - x[2bt]
    nc.vector.tensor_tensor(t4[:, 2], x3[:, 1:16:2], x3[:, 0:16:2], op=SUB)
    # r3 = x[2bt] - x[2bt+2]   (bt=7 edge: x[14])
    nc.vector.tensor_tensor(t4[:, 3, 0:7], x3[:, 0:14:2], x3[:, 2:16:2], op=SUB)
    nc.vector.tensor_copy(t4[:, 3, 7:8], x3[:, 14:15])
    # c-stage into the ring slot
    xs = xtv[:, slot]
    nc.vector.tensor_tensor(xs[:, :, 0, :, 1:8],
                            t4[:, :, :, 1:14:2], t4[:, :, :, 3:16:2], op=SUB)
    nc.vector.tensor_scalar_mul(xs[:, :, 0, :, 0:1], t4[:, :, :, 1:2], -1.0)
    nc.gpsimd.tensor_tensor(xs[:, :, 1],
                            t4[:, :, :, 0:16:2], t4[:, :, :, 1:16:2], op=ADD)
    nc.vector.tensor_tensor(xs[:, :, 2],
                            t4[:, :, :, 1:16:2], t4[:, :, :, 0:16:2], op=SUB)
    nc.vector.tensor_tensor(xs[:, :, 3, :, 0:7],
                            t4[:, :, :, 0:14:2], t4[:, :, :, 2:16:2], op=SUB)
    nc.vector.tensor_copy(xs[:, :, 3, :, 7:8], t4[:, :, :, 14:15])


def _inverse_c(nc, t3, tch, u):
    """c-stage of the inverse Winograd transform: T_b -> U spatial.

    t3: [128, rb2, k2, 512] where partition half h holds xc = 2k + h.
    tch: [64, 2048] staging — partition-remapped copy of t3's upper half
    (TensorTensor requires equal base partitions for two SBUF inputs, so
    the odd-xc data is first relocated to base 0 with a 1-input copy).
    u: [64, 2048] spatial sum, layout (bt, rb, ct, rc, j)."""
    # xc0=(k0,h0) xc1=(k0,h1) xc2=(k1,h0) xc3=(k1,h1)
    nc.vector.tensor_copy(tch[:], t3[64:128])
    tc3 = tch.rearrange("p (rb k f) -> p rb k f", rb=2, k=2, f=512)
    u4 = u.rearrange("p (b c j) -> p b c j", b=16, c=16, j=8)
    for rb in range(2):
        lo0, lo1 = t3[0:64, rb, 0], t3[0:64, rb, 1]
        hi0, hi1 = tc3[:, rb, 0], tc3[:, rb, 1]
        # rc=0: xc0 + xc1 + xc2
        dst = u4[:, rb:16:2, 0:16:2, :]
        nc.vector.tensor_tensor(dst, lo0, hi0, op=ADD)
        nc.gpsimd.tensor_tensor(dst, dst, lo1, op=ADD)
        # rc=1: xc1 - xc2 - xc3
        dst = u4[:, rb:16:2, 1:16:2, :]
        nc.vector.tensor_tensor(dst, hi0, lo1, op=SUB)
        nc.gpsimd.tensor_tensor(dst, dst, hi1, op=SUB)


@with_exitstack
def _conv_kernel(ctx, tc, la, slopes):
    """Emit the full 6-layer conv program. slopes: python floats len 5."""
    nc = tc.nc
    na4 = la // 4
    xcols = la * LB * 16 * J          # X'' free size (a, b, c, j)
    pcols = LB * CP * DP

    xin = nc.dram_tensor("xin", [2 * la, 4096], F32, kind="ExternalInput")
    w0_d = nc.dram_tensor("w0", [128, len(G_L0) * na4 * 128],
                          F16, kind="ExternalInput")
    wmid_d = [nc.dram_tensor(f"w{l}", [128, 16 * 3 * 64], F16,
                             kind="ExternalInput") for l in (1, 2, 3, 4)]
    w5_d = nc.dram_tensor("w5", [128, 16 * 3 * 4], F16, kind="ExternalInput")
    out_d = nc.dram_tensor("out_dec", [4, la * 2048], F16,
                           kind="ExternalOutput")

    const = ctx.enter_context(tc.tile_pool(name="const", bufs=1))
    pp = ctx.enter_context(tc.tile_pool(name="ps", bufs=4, space="PSUM"))

    # ---- persistent tiles ----
    w0t = const.tile([128, w0_d.shape[1]], F16, tag="w0")
    nc.sync.dma_start(w0t[:], w0_d[:])
    wmt = []
    for wd in wmid_d:
        t = const.tile([128, 16 * 3 * 64], F16, tag=wd.name)
        nc.sync.dma_start(t[:], wd[:])
        wmt.append(t)
    w5t = const.tile([128, 16 * 3 * 4], F16, tag="w5")
    nc.sync.dma_start(w5t[:], w5_d[:])

    xw = const.tile([128, xcols], F16, tag="xw")          # spatial X''
    xt = const.tile([128, 4 * 8192], F16, tag="xt")       # winograd ring
    xw4 = xw.rearrange("p (a b c j) -> p a b c j", a=la, b=LB, c=16, j=J)
    xtv = xt.rearrange("p (s xb xc bt ct j) -> p s xb xc bt ct j",
                       s=4, xb=4, xc=4, bt=8, ct=8, j=8)

    # zero the never-written d-edge slots once (s=0 j=0; s=3 j=7)
    nc.vector.memset(xw4[64:96, :, :, :, 0:1], 0.0)
    nc.gpsimd.memset(xw4[96:128, :, :, :, 7:8], 0.0)

    # ================= layer 0 =================
    with tc.tile_pool(name="l0", bufs=1) as l0p:
        xsb = l0p.tile([2 * la, 4096], F32, tag="xsb")
        nc.sync.dma_start(xsb[:], xin[:])
        xpad = l0p.tile([128, pcols], F16, tag="xpad")
        nc.vector.memset(xpad[:], 0.0)
        xp4 = xpad.rearrange("p (b c d) -> p b c d", b=LB, c=CP, d=DP)
        xs4 = xsb.rearrange("p (b c d) -> p b c d", b=LB, c=16, d=16)
        # block rg holds x shifted in d by dd = rg - 1 (zero-padded edges)
        for rg in range(3):
            dst = xp4[32 * rg:32 * rg + 2 * la, :, 1:17, 2 - rg:18 - rg]
            if rg == 1:
                nc.vector.tensor_copy(dst, xs4[:])
            else:
                nc.scalar.copy(dst, xs4[:])

        SMAP = {0: (1, 7, 1), 1: (0, 8, 0), 2: (0, 8, 1), 3: (0, 7, 2)}
        a_slope = slopes[0]
        for a0b in range(na4):
            if a0b == 1:
                # X'' slices 0..3 are complete: prime the L1 ring now so the
                # transforms overlap the rest of L0 on the PE
                for a2 in (0, 1, 2):
                    tbf0 = l0p.tile([128, 4096], F16, tag="tbf0", bufs=2)
                    _fwd_transform(nc, xw4, tbf0, xtv, a2 % 4, a2)
            for bc in range(8):          # b-pair chunks
                b0 = bc * 2
                ps = pp.tile([128, 512], F32, tag="ps")
                p4 = ps.rearrange("p (b c d) -> p b c d", b=2, c=16, d=16)
                for gi, (db, dc) in enumerate(G_L0):
                    blo = max(b0, -db)
                    bhi = min(b0 + 2, 16 - db)
                    cb = (gi * na4 + a0b) * 128
                    rhs = xp4[0:96, blo + db:bhi + db, dc + 1:dc + 17, 1:17]
                    out = p4[:, blo - b0:bhi - b0, :, :]
                    nc.tensor.matmul(out, w0t[0:96, cb:cb + 128], rhs,
                                     start=(gi == 0),
                                     stop=(gi == len(G_L0) - 1))
                sg = l0p.tile([128, 512], F16, tag="l0st", bufs=4)
                if USE_LRELU:
                    nc.scalar.activation(sg[:], ps[:],
                                         mybir.ActivationFunctionType.Lrelu,
                                         alpha=a_slope)
                else:
                    nc.scalar.activation(sg[:], ps[:],
                                         mybir.ActivationFunctionType.Relu,
                                         scale=1.0 - a_slope)
                    nc.vector.scalar_tensor_tensor(
                        sg[:], ps[:], a_slope, sg[:],
                        op0=mybir.AluOpType.mult, op1=mybir.AluOpType.add)
                sg4 = sg.rearrange("p (b c d) -> p b c d", b=2, c=16, d=16)
                for aj in range(4):
                    a = a0b * 4 + aj
                    for s in range(4):
                        jlo, jcnt, dlo = SMAP[s]
                        dst = xw4[PHYS[s]:PHYS[s] + 32, a, b0:b0 + 2, :,
                                  jlo:jlo + jcnt]
                        src = sg4[32 * aj:32 * aj + 32, :, :,
                                  dlo:dlo + 2 * jcnt - 1:2]
                        if s in (0, 2):
                            nc.scalar.copy(dst, src)
                        else:
                            nc.vector.tensor_copy(dst, src)

    stg = ctx.enter_context(tc.tile_pool(name="stg", bufs=2))

    def _transform(a2):
        tbf = stg.tile([128, 4096], F16, tag="tbf")
        _fwd_transform(nc, xw4, tbf, xtv, a2 % 4, a2)

    # ================= layers 1..5 (winograd b,c) =================
    for l in range(1, 6):
        is_l5 = (l == 5)
        wt = w5t if is_l5 else wmt[l - 1]
        a_slope = slopes[l] if not is_l5 else 0.0

        def _inverse_chain(pa, s3, t3, u, is_l5=is_l5, a_slope=a_slope):
            """Deferred b-inverse/c-inverse/PReLU/scatter for slice pa."""
            nc.vector.tensor_tensor(t3[:, 0], s3[:, 0:2], s3[:, 2:4], op=ADD)
            nc.vector.tensor_tensor(t3[:, 0], t3[:, 0], s3[:, 4:6], op=ADD)
            nc.vector.tensor_tensor(t3[:, 1], s3[:, 2:4], s3[:, 4:6], op=SUB)
            nc.vector.tensor_tensor(t3[:, 1], t3[:, 1], s3[:, 6:8], op=SUB)
            tch = stg.tile([64, 2048], F16, tag="tch", bufs=1)
            _inverse_c(nc, t3, tch, u)
            if is_l5:
                nc.sync.dma_start(out_d[:, pa * 2048:(pa + 1) * 2048],
                                  u[0:4, :])
                return
            # PReLU into the direct d-slots s'=1,2
            dst = xw4[0:64, pa]
            if USE_LRELU:
                nc.scalar.activation(dst, u[:],
                                     mybir.ActivationFunctionType.Lrelu,
                                     alpha=a_slope)
            else:
                nc.scalar.activation(dst, u[:],
                                     mybir.ActivationFunctionType.Relu,
                                     scale=1.0 - a_slope)
                nc.vector.scalar_tensor_tensor(
                    dst, u[:], a_slope, dst,
                    op0=mybir.AluOpType.mult, op1=mybir.AluOpType.add)
            # j-shift copies: s'=3 <- s'=1 (j+1); s'=0 <- s'=2 (j-1)
            nc.vector.tensor_copy(xw4[96:128, pa, :, :, 0:7],
                                  xw4[0:32, pa, :, :, 1:8])
            nc.vector.tensor_copy(xw4[64:96, pa, :, :, 1:8],
                                  xw4[32:64, pa, :, :, 0:7])

        pending = None
        for a in range(la):
            # ---- forward-transform slice a+2 (slices 0..2 primed) ----
            if 1 <= a and a + 2 < la:
                _transform(a + 2)

            # ---- matmuls: accumulate 3 da taps per xi (dai-outer so the
            # oldest ring slot is released early) ----
            dais = [d for d in (0, 1, 2) if 0 <= a + d - 1 < la]
            sev = stg.tile([128, 4096], F16, tag="sev")
            s3 = sev.rearrange("p (q f) -> p q f", q=8, f=512)
            tbi = stg.tile([128, 2048], F16, tag="tbi", bufs=1)
            t3 = tbi.rearrange("p (rb k f) -> p rb k f", rb=2, k=2, f=512)
            u = stg.tile([64, 2048], F16, tag="u", bufs=1)
            prev = None
            if is_l5:
                # 8 xi per 2-bank tile: xi = tau*8 + k*4 + q, rows 32q, bank k
                pst = [pp.tile([128, 1024], F32, tag="ps", name=f"p5{t_}",
                               bufs=4) for t_ in range(2)]
                for idx, dai in enumerate(dais):
                    slot = (a + dai - 1) % 4
                    for xi in range(16):
                        tau, r = divmod(xi, 8)
                        k, q = divmod(r, 4)
                        rhs = xt[:, slot * 8192 + xi * 512:
                                 slot * 8192 + xi * 512 + 512]
                        out = pst[tau][32 * q:32 * q + 4,
                                       512 * k:512 * k + 512]
                        cb = (xi * 3 + dai) * 4
                        mm = nc.tensor.matmul(out, wt[:, cb:cb + 4], rhs,
                                              start=(idx == 0),
                                              stop=(idx == len(dais) - 1),
                                              tile_position=(0, 32 * q))
                        if prev is not None:
                            tile.add_dep_helper(mm.ins, prev.ins,
                                                sync=False, reason="order")
                        prev = mm
                for xi in range(16):
                    tau, r = divmod(xi, 8)
                    k, qq = divmod(r, 4)
                    h, q = (xi % 4) % 2, 2 * (xi // 4) + (xi % 4) // 2
                    src = pst[tau][32 * qq:32 * qq + 4, 512 * k:512 * k + 512]
                    nc.scalar.copy(s3[64 * h:64 * h + 4, q], src)
            else:
                # two 2-bank psum tiles per xb pair: halves hold (k = xc//2)
                pst = [pp.tile([128, 1024], F32, tag="ps", name=f"pm{q}",
                               bufs=4) for q in range(4)]
                for idx, dai in enumerate(dais):
                    slot = (a + dai - 1) % 4
                    for xi in range(16):
                        xb, xc = divmod(xi, 4)
                        h, k = xc % 2, xc // 2
                        rhs = xt[:, slot * 8192 + xi * 512:
                                 slot * 8192 + xi * 512 + 512]
                        out = pst[xb][64 * h:64 * h + 64,
                                      512 * k:512 * k + 512]
                        cb = (xi * 3 + dai) * 64
                        mm = nc.tensor.matmul(out, wt[:, cb:cb + 64], rhs,
                                              start=(idx == 0),
                                              stop=(idx == len(dais) - 1),
                                              tile_position=(0, 64 * h))
                        if prev is not None:
                            tile.add_dep_helper(mm.ins, prev.ins,
                                                sync=False, reason="order")
                        prev = mm
                # S free layout q = 2*xb + k -> one 2-bank evict per xb
                for xb in range(4):
                    nc.scalar.copy(sev[:, xb * 1024:(xb + 1) * 1024],
                                   pst[xb][:])

            # ---- deferred inverse chain of the previous slice ----
            if pending is not None:
                _inverse_chain(*pending)
            pending = (a, s3, t3, u)

            # ---- hoist the next layer's ring primes into this one ----
            if a >= la - 3 and l < 5:
                _transform(a - (la - 3))

        _inverse_chain(*pending)


_CACHE = {}
LAST_RESULT = None


def _build(la, slopes):
    key = (la, tuple(slopes))
    if key not in _CACHE:
        nc = bacc.Bacc("TRN2")
        with tile.TileContext(nc) as tc:
            _conv_kernel(tc, la, slopes)
        nc.compile()
        _CACHE[key] = nc
    return _CACHE[key]


def kernel(x, k0, k1, k2, k3, k4, k5, slopes):
    x = np.asarray(x, np.float32)
    n, _, la = x.shape[:3]
    slopes_f = [float(s) for s in np.asarray(slopes, np.float32)]
    ws = _pack_weights((k0, k1, k2, k3, k4, k5), la)
    nc = _build(la, slopes_f)

    in_maps = []
    for i in range(n):
        m = {"xin": np.ascontiguousarray(x[i].reshape(2 * la, 4096)),
             "w0": ws[0], "w5": ws[5]}
        for l in (1, 2, 3, 4):
            m[f"w{l}"] = ws[l]
        in_maps.append(m)

    res = bass_utils.run_bass_kernel_spmd(nc, in_maps,
                                          core_ids=list(range(n)))
    global LAST_RESULT
    LAST_RESULT = res
    outs = []
    for i in range(n):
        od = np.asarray(res.results[i]["out_dec"], np.float32)
        od = od.reshape(2, 2, la, 16, 16, 8)
        # [o, t, a, b, c, j] -> [o, a, b, c, j, t] -> d = 2j + t
        o = np.transpose(od, (0, 2, 3, 4, 5, 1)).reshape(2, la, 16, 16, 16)
        outs.append(o)
    return np.stack(outs).astype(np.float32)


# revision 27
# speedup vs baseline: 1.7090x; 1.1322x over previous
"""Trainium2 Bass kernel for a 6-layer 4D CNN (3^4 SAME convs + PReLU).

Problem: x (8, 2, 16,16,16,16) -> 6 conv layers, channels 2->32->32->32->32
->32->2, PReLU (scalar slope) after the first five convs.

Strategy (per NeuronCore, data-parallel over batch N=8 across 8 cores):
  * d-axis banded-Toeplitz matmuls: activations live in SBUF in a
    "decimated" layout X''[32*s + ch, (a, b, c, j)] where block s in 0..3
    holds d = 2*j + s - 1 (d-phases).  One matmul contracts
    K = 128 = (4 d-phases x 32 ch) and produces M = 64 = (2 d-outs x 32 ch)
    outputs per column: the 3-tap d-convolution is folded into the
    stationary (block-banded) weight matrix.
  * Winograd F(2,3)^2 over the (b, c) axes for layers 1..5: the 9 (db, dc)
    taps become 16 independent transformed points (xi_b, xi_c in 0..3) with
    only the 3 da taps left as PSUM accumulation.  Streamed matmul columns
    per layer drop from 27 * (out/64) to 16 * 3 * (out/128): 2.25x less
    TensorE time.  Forward/inverse transforms are +-1 adds done on the
    Vector/GpSimd engines; PReLU (ACT Lrelu) applies in the spatial domain
    between inverse and the next forward transform.
  * Layer 0 (2 input channels) uses an a-partition scheme: partitions =
    (ch, a), M = 128 = (4 a-outs x 32 ch), a-banded stationaries; the dd
    taps are folded into K = 96 via three d-shifted input replicas, so the
    27 taps collapse to 9 (db, dc) matmul groups.
  * Layer 5 (2 output channels) reuses the Winograd path with M = 4
    (2 ch x 2 d-outs) packed 4-per-PSUM-bank; result leaves in decimated
    (o, t) layout, host reassembles d = 2j + t.
  * fp16 activations and weights, fp32 PSUM accumulation.
"""

import sys

import numpy as np

for _p in ("/opt/trn_rl_repo", "/root/.axon_site/_ro/trn_rl_repo"):
    if _p not in sys.path:
        sys.path.append(_p)

import concourse.bass as bass  # noqa: E402
import concourse.mybir as mybir  # noqa: E402
import concourse.tile as tile  # noqa: E402
from concourse import bacc, bass_utils  # noqa: E402
from concourse._compat import with_exitstack  # noqa: E402

F32 = mybir.dt.float32
F16 = mybir.dt.float16
ADD = mybir.AluOpType.add
SUB = mybir.AluOpType.subtract

LB = 16
CP = 18   # padded c axis for the L0 input replica
DP = 18   # padded d axis in x_pad
J = 8     # d//2

USE_LRELU = True

# L0 groups: (db, dc) only — the dd taps are folded into K=96 via three
# d-shifted partition-block replicas of the input
G_L0 = [(db, dc) for db in (0, -1, 1) for dc in (0, -1, 1)]

# physical partition offset of logical d-phase block s in X''/X-tilde:
# direct PReLU outputs (s=1,2) sit at base 0 so the [64]-partition ACT
# write is 64-aligned (HW: >32-partition access must be 64-aligned)
PHYS = {0: 64, 1: 0, 2: 32, 3: 96}

# Winograd F(2,3) matrices (cross-correlation form)
_G = np.array([[1, 0, 0], [.5, .5, .5], [.5, -.5, .5], [0, 0, 1]], np.float32)


def _pack_weights(ks, la):
    """Host-side packing of conv kernels into stationary matrices (fp16)."""
    na4 = la // 4
    k0, k1, k2, k3, k4, k5 = [np.asarray(k, np.float32) for k in ks]

    # L0: W0[32*rg + i*la + a_in, (g*na4 + a0b)*128 + a_j*32 + o]
    w0 = np.zeros((128, len(G_L0) * na4 * 128), np.float32)
    for gi, (db, dc) in enumerate(G_L0):
        for a0b in range(na4):
            cb = (gi * na4 + a0b) * 128
            for rg in range(3):
                for aj in range(4):
                    for da in (-1, 0, 1):
                        ain = a0b * 4 + aj + da
                        if not (0 <= ain < la):
                            continue
                        for i in range(2):
                            w0[32 * rg + i * la + ain,
                               cb + aj * 32:cb + aj * 32 + 32] = \
                                k0[:, i, da + 1, db + 1, dc + 1, rg]

    # winograd-transformed mid layers:
    # khat[o,i,da,xb,xc,dd] = sum_{db,dc} G[xb,db] G[xc,dc] k[o,i,da,db,dc,dd]
    # W[32s+i, ((xb*4+xc)*3 + dai)*64 + t*32 + o] = khat[o,i,dai,xb,xc,s-t]
    def pack_mid_wino(k):
        kh = np.einsum("up,vq,oiapqd->oiauvd", _G, _G, k, optimize=True)
        w = np.zeros((128, 16 * 3 * 64), np.float32)
        for xb in range(4):
            for xc in range(4):
                for dai in range(3):
                    cb = (((xb * 4 + xc) * 3) + dai) * 64
                    for s in range(4):
                        for t in range(2):
                            if 0 <= s - t <= 2:
                                w[PHYS[s]:PHYS[s] + 32,
                                  cb + t * 32:cb + t * 32 + 32] = \
                                    kh[:, :, dai, xb, xc, s - t].T
        return w

    # L5 winograd: W5[32s+i, ((xb*4+xc)*3 + dai)*4 + o*2 + t]
    kh5 = np.einsum("up,vq,oiapqd->oiauvd", _G, _G, k5, optimize=True)
    w5 = np.zeros((128, 16 * 3 * 4), np.float32)
    for xb in range(4):
        for xc in range(4):
            for dai in range(3):
                cb = (((xb * 4 + xc) * 3) + dai) * 4
                for s in range(4):
                    for t in range(2):
                        if 0 <= s - t <= 2:
                            for o in range(2):
                                w5[PHYS[s]:PHYS[s] + 32, cb + o * 2 + t] = \
                                    kh5[o, :, dai, xb, xc, s - t]

    return ([w0.astype(np.float16)] +
            [pack_mid_wino(k).astype(np.float16) for k in (k1, k2, k3, k4)] +
            [w5.astype(np.float16)])


def _fwd_transform(nc, xw4, tbf, xtv, slot, a):
    """Forward Winograd transform of X'' slice a -> ring slot.

    xw4: X'' view [128, a, b16, c16, j8]; tbf: [128, 4096] staging;
    xtv: ring view [128, slot, xb, xc, bt, ct, j]."""
    t4 = tbf.rearrange("p (xb bt c j) -> p xb bt c j", xb=4, bt=8, c=16, j=8)
    x3 = xw4[:, a]
    # b-stage: B^T rows over b-windows 2bt-1..2bt+2
    # r0 = x[2bt-1] - x[2bt+1]   (bt=0 edge: -x[1])
    nc.vector.tensor_tensor(t4[:, 0, 1:8], x3[:, 1:14:2], x3[:, 3:16:2], op=SUB)
    nc.vector.tensor_scalar_mul(t4[:, 0, 0:1], x3[:, 1:2], -1.0)
    # r1 = x[2bt] + x[2bt+1]
    nc.vector.tensor_tensor(t4[:, 1], x3[:, 0:16:2], x3[:, 1:16:2], op=ADD)
    # r2 = x[2bt+1] - x[2bt]
    nc.vector.tensor_tensor(t4[:, 2], x3[:, 1:16:2], x3[:, 0:16:2], op=SUB)
    # r3 = x[2bt] - x[2bt+2]   (bt=7 edge: x[14])
    nc.vector.tensor_tensor(t4[:, 3, 0:7], x3[:, 0:14:2], x3[:, 2:16:2], op=SUB)
    nc.vector.tensor_copy(t4[:, 3, 7:8], x3[:, 14:15])
    # c-stage into the ring slot
    xs = xtv[:, slot]
    nc.vector.tensor_tensor(xs[:, :, 0, :, 1:8],
                            t4[:, :, :, 1:14:2], t4[:, :, :, 3:16:2], op=SUB)
    nc.vector.tensor_scalar_mul(xs[:, :, 0, :, 0:1], t4[:, :, :, 1:2], -1.0)
    nc.gpsimd.tensor_tensor(xs[:, :, 1],
                            t4[:, :, :, 0:16:2], t4[:, :, :, 1:16:2], op=ADD)
    nc.vector.tensor_tensor(xs[:, :, 2],
                            t4[:, :, :, 1:16:2], t4[:, :, :, 0:16:2], op=SUB)
    nc.vector.tensor_tensor(xs[:, :, 3, :, 0:7],
                            t4[:, :, :, 0:14:2], t4[:, :, :, 2:16:2], op=SUB)
    nc.vector.tensor_copy(xs[:, :, 3, :, 7:8], t4[:, :, :, 14:15])


def _inverse_c(nc, t3, tch, u):
    """c-stage of the inverse Winograd transform: T_b -> U spatial.

    t3: [128, rb2, k2, 512] where partition half h holds xc = 2k + h.
    tch: [64, 2048] staging — partition-remapped copy of t3's upper half
    (TensorTensor requires equal base partitions for two SBUF inputs, so
    the odd-xc data is first relocated to base 0 with a 1-input copy).
    u: [64, 2048] spatial sum, layout (bt, rb, ct, rc, j)."""
    # xc0=(k0,h0) xc1=(k0,h1) xc2=(k1,h0) xc3=(k1,h1)
    nc.vector.tensor_copy(tch[:], t3[64:128])
    tc3 = tch.rearrange("p (rb k f) -> p rb k f", rb=2, k=2, f=512)
    u4 = u.rearrange("p (b c j) -> p b c j", b=16, c=16, j=8)
    for rb in range(2):
        lo0, lo1 = t3[0:64, rb, 0], t3[0:64, rb, 1]
        hi0, hi1 = tc3[:, rb, 0], tc3[:, rb, 1]
        # rc=0: xc0 + xc1 + xc2
        dst = u4[:, rb:16:2, 0:16:2, :]
        nc.vector.tensor_tensor(dst, lo0, hi0, op=ADD)
        nc.vector.tensor_tensor(dst, dst, lo1, op=ADD)
        # rc=1: xc1 - xc2 - xc3
        dst = u4[:, rb:16:2, 1:16:2, :]
        nc.vector.tensor_tensor(dst, hi0, lo1, op=SUB)
        nc.vector.tensor_tensor(dst, dst, hi1, op=SUB)


@with_exitstack
def _conv_kernel(ctx, tc, la, slopes):
    """Emit the full 6-layer conv program. slopes: python floats len 5."""
    nc = tc.nc
    na4 = la // 4
    xcols = la * LB * 16 * J          # X'' free size (a, b, c, j)
    pcols = LB * CP * DP

    xin = nc.dram_tensor("xin", [2 * la, 4096], F32, kind="ExternalInput")
    w0_d = nc.dram_tensor("w0", [128, len(G_L0) * na4 * 128],
                          F16, kind="ExternalInput")
    wmid_d = [nc.dram_tensor(f"w{l}", [128, 16 * 3 * 64], F16,
                             kind="ExternalInput") for l in (1, 2, 3, 4)]
    w5_d = nc.dram_tensor("w5", [128, 16 * 3 * 4], F16, kind="ExternalInput")
    out_d = nc.dram_tensor("out_dec", [4, la * 2048], F16,
                           kind="ExternalOutput")

    const = ctx.enter_context(tc.tile_pool(name="const", bufs=1))
    pp = ctx.enter_context(tc.tile_pool(name="ps", bufs=4, space="PSUM"))

    # ---- persistent tiles ----
    w0t = const.tile([128, w0_d.shape[1]], F16, tag="w0")
    nc.sync.dma_start(w0t[:], w0_d[:])
    wmt = []
    for wd in wmid_d:
        t = const.tile([128, 16 * 3 * 64], F16, tag=wd.name)
        nc.sync.dma_start(t[:], wd[:])
        wmt.append(t)
    w5t = const.tile([128, 16 * 3 * 4], F16, tag="w5")
    nc.sync.dma_start(w5t[:], w5_d[:])

    xw = const.tile([128, xcols], F16, tag="xw")          # spatial X''
    xt = const.tile([128, 4 * 8192], F16, tag="xt")       # winograd ring
    xw4 = xw.rearrange("p (a b c j) -> p a b c j", a=la, b=LB, c=16, j=J)
    xtv = xt.rearrange("p (s xb xc bt ct j) -> p s xb xc bt ct j",
                       s=4, xb=4, xc=4, bt=8, ct=8, j=8)

    # zero the never-written d-edge slots once (s=0 j=0; s=3 j=7)
    nc.vector.memset(xw4[64:96, :, :, :, 0:1], 0.0)
    nc.gpsimd.memset(xw4[96:128, :, :, :, 7:8], 0.0)

    # ================= layer 0 =================
    with tc.tile_pool(name="l0", bufs=1) as l0p:
        xsb = l0p.tile([2 * la, 4096], F32, tag="xsb")
        nc.sync.dma_start(xsb[:], xin[:])
        xpad = l0p.tile([128, pcols], F16, tag="xpad")
        nc.vector.memset(xpad[:], 0.0)
        xp4 = xpad.rearrange("p (b c d) -> p b c d", b=LB, c=CP, d=DP)
        xs4 = xsb.rearrange("p (b c d) -> p b c d", b=LB, c=16, d=16)
        # block rg holds x shifted in d by dd = rg - 1 (zero-padded edges)
        for rg in range(3):
            dst = xp4[32 * rg:32 * rg + 2 * la, :, 1:17, 2 - rg:18 - rg]
            if rg == 1:
                nc.vector.tensor_copy(dst, xs4[:])
            else:
                nc.scalar.copy(dst, xs4[:])

        SMAP = {0: (1, 7, 1), 1: (0, 8, 0), 2: (0, 8, 1), 3: (0, 7, 2)}
        a_slope = slopes[0]
        for a0b in range(na4):
            if a0b == 1:
                # X'' slices 0..3 are complete: prime the L1 ring now so the
                # transforms overlap the rest of L0 on the PE
                for a2 in (0, 1, 2):
                    tbf0 = l0p.tile([128, 4096], F16, tag="tbf0", bufs=2)
                    _fwd_transform(nc, xw4, tbf0, xtv, a2 % 4, a2)
            for bc in range(8):          # b-pair chunks
                b0 = bc * 2
                ps = pp.tile([128, 512], F32, tag="ps")
                p4 = ps.rearrange("p (b c d) -> p b c d", b=2, c=16, d=16)
                for gi, (db, dc) in enumerate(G_L0):
                    blo = max(b0, -db)
                    bhi = min(b0 + 2, 16 - db)
                    cb = (gi * na4 + a0b) * 128
                    rhs = xp4[0:96, blo + db:bhi + db, dc + 1:dc + 17, 1:17]
                    out = p4[:, blo - b0:bhi - b0, :, :]
                    nc.tensor.matmul(out, w0t[0:96, cb:cb + 128], rhs,
                                     start=(gi == 0),
                                     stop=(gi == len(G_L0) - 1))
                sg = l0p.tile([128, 512], F16, tag="l0st", bufs=4)
                if USE_LRELU:
                    nc.scalar.activation(sg[:], ps[:],
                                         mybir.ActivationFunctionType.Lrelu,
                                         alpha=a_slope)
                else:
                    nc.scalar.activation(sg[:], ps[:],
                                         mybir.ActivationFunctionType.Relu,
                                         scale=1.0 - a_slope)
                    nc.vector.scalar_tensor_tensor(
                        sg[:], ps[:], a_slope, sg[:],
                        op0=mybir.AluOpType.mult, op1=mybir.AluOpType.add)
                sg4 = sg.rearrange("p (b c d) -> p b c d", b=2, c=16, d=16)
                for aj in range(4):
                    a = a0b * 4 + aj
                    for s in range(4):
                        jlo, jcnt, dlo = SMAP[s]
                        dst = xw4[PHYS[s]:PHYS[s] + 32, a, b0:b0 + 2, :,
                                  jlo:jlo + jcnt]
                        src = sg4[32 * aj:32 * aj + 32, :, :,
                                  dlo:dlo + 2 * jcnt - 1:2]
                        if s in (0, 2):
                            nc.scalar.copy(dst, src)
                        else:
                            nc.vector.tensor_copy(dst, src)

    stg = ctx.enter_context(tc.tile_pool(name="stg", bufs=2))

    def _transform(a2):
        tbf = stg.tile([128, 4096], F16, tag="tbf")
        _fwd_transform(nc, xw4, tbf, xtv, a2 % 4, a2)

    # ================= layers 1..5 (winograd b,c) =================
    for l in range(1, 6):
        is_l5 = (l == 5)
        wt = w5t if is_l5 else wmt[l - 1]
        a_slope = slopes[l] if not is_l5 else 0.0

        def _inverse_chain(pa, s3, t3, u, is_l5=is_l5, a_slope=a_slope):
            """Deferred b-inverse/c-inverse/PReLU/scatter for slice pa."""
            nc.vector.tensor_tensor(t3[:, 0], s3[:, 0:2], s3[:, 2:4], op=ADD)
            nc.vector.tensor_tensor(t3[:, 0], t3[:, 0], s3[:, 4:6], op=ADD)
            nc.vector.tensor_tensor(t3[:, 1], s3[:, 2:4], s3[:, 4:6], op=SUB)
            nc.vector.tensor_tensor(t3[:, 1], t3[:, 1], s3[:, 6:8], op=SUB)
            tch = stg.tile([64, 2048], F16, tag="tch", bufs=1)
            _inverse_c(nc, t3, tch, u)
            if is_l5:
                nc.sync.dma_start(out_d[:, pa * 2048:(pa + 1) * 2048],
                                  u[0:4, :])
                return
            # PReLU into the direct d-slots s'=1,2
            dst = xw4[0:64, pa]
            if USE_LRELU:
                nc.scalar.activation(dst, u[:],
                                     mybir.ActivationFunctionType.Lrelu,
                                     alpha=a_slope)
            else:
                nc.scalar.activation(dst, u[:],
                                     mybir.ActivationFunctionType.Relu,
                                     scale=1.0 - a_slope)
                nc.vector.scalar_tensor_tensor(
                    dst, u[:], a_slope, dst,
                    op0=mybir.AluOpType.mult, op1=mybir.AluOpType.add)
            # j-shift copies: s'=3 <- s'=1 (j+1); s'=0 <- s'=2 (j-1)
            nc.gpsimd.tensor_copy(xw4[96:128, pa, :, :, 0:7],
                                  xw4[0:32, pa, :, :, 1:8])
            nc.gpsimd.tensor_copy(xw4[64:96, pa, :, :, 1:8],
                                  xw4[32:64, pa, :, :, 0:7])

        pending = None
        for a in range(la):
            # ---- forward-transform slice a+2 (slices 0..2 primed) ----
            if 1 <= a and a + 2 < la:
                _transform(a + 2)

            # ---- matmuls: accumulate 3 da taps per xi (dai-outer so the
            # oldest ring slot is released early) ----
            dais = [d for d in (0, 1, 2) if 0 <= a + d - 1 < la]
            sev = stg.tile([128, 4096], F16, tag="sev")
            s3 = sev.rearrange("p (q f) -> p q f", q=8, f=512)
            tbi = stg.tile([128, 2048], F16, tag="tbi", bufs=1)
            t3 = tbi.rearrange("p (rb k f) -> p rb k f", rb=2, k=2, f=512)
            u = stg.tile([64, 2048], F16, tag="u", bufs=1)
            prev = None
            if is_l5:
                # 8 xi per 2-bank tile: xi = tau*8 + k*4 + q, rows 32q, bank k
                pst = [pp.tile([128, 1024], F32, tag="ps", name=f"p5{t_}",
                               bufs=4) for t_ in range(2)]
                for idx, dai in enumerate(dais):
                    slot = (a + dai - 1) % 4
                    for xi in range(16):
                        tau, r = divmod(xi, 8)
                        k, q = divmod(r, 4)
                        rhs = xt[:, slot * 8192 + xi * 512:
                                 slot * 8192 + xi * 512 + 512]
                        out = pst[tau][32 * q:32 * q + 4,
                                       512 * k:512 * k + 512]
                        cb = (xi * 3 + dai) * 4
                        mm = nc.tensor.matmul(out, wt[:, cb:cb + 4], rhs,
                                              start=(idx == 0),
                                              stop=(idx == len(dais) - 1),
                                              tile_position=(0, 32 * q))
                        if prev is not None:
                            tile.add_dep_helper(mm.ins, prev.ins,
                                                sync=False, reason="order")
                        prev = mm
                for xi in range(16):
                    tau, r = divmod(xi, 8)
                    k, qq = divmod(r, 4)
                    h, q = (xi % 4) % 2, 2 * (xi // 4) + (xi % 4) // 2
                    src = pst[tau][32 * qq:32 * qq + 4, 512 * k:512 * k + 512]
                    nc.scalar.copy(s3[64 * h:64 * h + 4, q], src)
            else:
                # two 2-bank psum tiles per xb pair: halves hold (k = xc//2)
                pst = [pp.tile([128, 1024], F32, tag="ps", name=f"pm{q}",
                               bufs=4) for q in range(4)]
                for idx, dai in enumerate(dais):
                    slot = (a + dai - 1) % 4
                    for xi in range(16):
                        xb, xc = divmod(xi, 4)
                        h, k = xc % 2, xc // 2
                        rhs = xt[:, slot * 8192 + xi * 512:
                                 slot * 8192 + xi * 512 + 512]
                        out = pst[xb][64 * h:64 * h + 64,
                                      512 * k:512 * k + 512]
                        cb = (xi * 3 + dai) * 64
                        mm = nc.tensor.matmul(out, wt[:, cb:cb + 64], rhs,
                                              start=(idx == 0),
                                              stop=(idx == len(dais) - 1),
                                              tile_position=(0, 64 * h))
                        if prev is not None:
                            tile.add_dep_helper(mm.ins, prev.ins,
                                                sync=False, reason="order")
                        prev = mm
                # S free layout q = 2*xb + k -> one 2-bank evict per xb
                for xb in range(4):
                    nc.scalar.copy(sev[:, xb * 1024:(xb + 1) * 1024],
                                   pst[xb][:])

            # ---- deferred inverse chain of the previous slice ----
            if pending is not None:
                _inverse_chain(*pending)
            pending = (a, s3, t3, u)

            # ---- hoist the next layer's ring primes into this one ----
            if a >= la - 3 and l < 5:
                _transform(a - (la - 3))

        _inverse_chain(*pending)


_CACHE = {}
LAST_RESULT = None


def _build(la, slopes):
    key = (la, tuple(slopes))
    if key not in _CACHE:
        nc = bacc.Bacc("TRN2")
        with tile.TileContext(nc) as tc:
            _conv_kernel(tc, la, slopes)
        nc.compile()
        _CACHE[key] = nc
    return _CACHE[key]


def kernel(x, k0, k1, k2, k3, k4, k5, slopes):
    x = np.asarray(x, np.float32)
    n, _, la = x.shape[:3]
    slopes_f = [float(s) for s in np.asarray(slopes, np.float32)]
    ws = _pack_weights((k0, k1, k2, k3, k4, k5), la)
    nc = _build(la, slopes_f)

    in_maps = []
    for i in range(n):
        m = {"xin": np.ascontiguousarray(x[i].reshape(2 * la, 4096)),
             "w0": ws[0], "w5": ws[5]}
        for l in (1, 2, 3, 4):
            m[f"w{l}"] = ws[l]
        in_maps.append(m)

    res = bass_utils.run_bass_kernel_spmd(nc, in_maps,
                                          core_ids=list(range(n)))
    global LAST_RESULT
    LAST_RESULT = res
    outs = []
    for i in range(n):
        od = np.asarray(res.results[i]["out_dec"], np.float32)
        od = od.reshape(2, 2, la, 16, 16, 8)
        # [o, t, a, b, c, j] -> [o, a, b, c, j, t] -> d = 2j + t
        o = np.transpose(od, (0, 2, 3, 4, 5, 1)).reshape(2, la, 16, 16, 16)
        outs.append(o)
    return np.stack(outs).astype(np.float32)


# revision 28
# speedup vs baseline: 1.7562x; 1.0277x over previous
"""Trainium2 Bass kernel for a 6-layer 4D CNN (3^4 SAME convs + PReLU).

Problem: x (8, 2, 16,16,16,16) -> 6 conv layers, channels 2->32->32->32->32
->32->2, PReLU (scalar slope) after the first five convs.

Strategy (per NeuronCore, data-parallel over batch N=8 across 8 cores):
  * d-axis banded-Toeplitz matmuls: activations live in SBUF in a
    "decimated" layout X''[32*s + ch, (a, b, c, j)] where block s in 0..3
    holds d = 2*j + s - 1 (d-phases).  One matmul contracts
    K = 128 = (4 d-phases x 32 ch) and produces M = 64 = (2 d-outs x 32 ch)
    outputs per column: the 3-tap d-convolution is folded into the
    stationary (block-banded) weight matrix.
  * Winograd F(2,3)^2 over the (b, c) axes for layers 1..5: the 9 (db, dc)
    taps become 16 independent transformed points (xi_b, xi_c in 0..3) with
    only the 3 da taps left as PSUM accumulation.  Streamed matmul columns
    per layer drop from 27 * (out/64) to 16 * 3 * (out/128): 2.25x less
    TensorE time.  Forward/inverse transforms are +-1 adds done on the
    Vector/GpSimd engines; PReLU (ACT Lrelu) applies in the spatial domain
    between inverse and the next forward transform.
  * Layer 0 (2 input channels) uses an a-partition scheme: partitions =
    (ch, a), M = 128 = (4 a-outs x 32 ch), a-banded stationaries; the dd
    taps are folded into K = 96 via three d-shifted input replicas, so the
    27 taps collapse to 9 (db, dc) matmul groups.
  * Layer 5 (2 output channels) reuses the Winograd path with M = 4
    (2 ch x 2 d-outs) packed 4-per-PSUM-bank; result leaves in decimated
    (o, t) layout, host reassembles d = 2j + t.
  * fp16 activations and weights, fp32 PSUM accumulation.
"""

import sys

import numpy as np

for _p in ("/opt/trn_rl_repo", "/root/.axon_site/_ro/trn_rl_repo"):
    if _p not in sys.path:
        sys.path.append(_p)

import concourse.bass as bass  # noqa: E402
import concourse.mybir as mybir  # noqa: E402
import concourse.tile as tile  # noqa: E402
from concourse import bacc, bass_utils  # noqa: E402
from concourse._compat import with_exitstack  # noqa: E402

F32 = mybir.dt.float32
F16 = mybir.dt.float16
ADD = mybir.AluOpType.add
SUB = mybir.AluOpType.subtract

LB = 16
CP = 18   # padded c axis for the L0 input replica
DP = 18   # padded d axis in x_pad
J = 8     # d//2

USE_LRELU = True

# L0 groups: (db, dc) only — the dd taps are folded into K=96 via three
# d-shifted partition-block replicas of the input
G_L0 = [(db, dc) for db in (0, -1, 1) for dc in (0, -1, 1)]

# physical partition offset of logical d-phase block s in X''/X-tilde:
# direct PReLU outputs (s=1,2) sit at base 0 so the [64]-partition ACT
# write is 64-aligned (HW: >32-partition access must be 64-aligned)
PHYS = {0: 64, 1: 0, 2: 32, 3: 96}

# Winograd F(2,3) matrices (cross-correlation form)
_G = np.array([[1, 0, 0], [.5, .5, .5], [.5, -.5, .5], [0, 0, 1]], np.float32)


def _pack_weights(ks, la):
    """Host-side packing of conv kernels into stationary matrices (fp16)."""
    na4 = la // 4
    k0, k1, k2, k3, k4, k5 = [np.asarray(k, np.float32) for k in ks]

    # L0: W0[32*rg + i*la + a_in, (g*na4 + a0b)*128 + a_j*32 + o]
    w0 = np.zeros((128, len(G_L0) * na4 * 128), np.float32)
    for gi, (db, dc) in enumerate(G_L0):
        for a0b in range(na4):
            cb = (gi * na4 + a0b) * 128
            for rg in range(3):
                for aj in range(4):
                    for da in (-1, 0, 1):
                        ain = a0b * 4 + aj + da
                        if not (0 <= ain < la):
                            continue
                        for i in range(2):
                            w0[32 * rg + i * la + ain,
                               cb + aj * 32:cb + aj * 32 + 32] = \
                                k0[:, i, da + 1, db + 1, dc + 1, rg]

    # winograd-transformed mid layers:
    # khat[o,i,da,xb,xc,dd] = sum_{db,dc} G[xb,db] G[xc,dc] k[o,i,da,db,dc,dd]
    # W[32s+i, ((xb*4+xc)*3 + dai)*64 + t*32 + o] = khat[o,i,dai,xb,xc,s-t]
    def pack_mid_wino(k):
        kh = np.einsum("up,vq,oiapqd->oiauvd", _G, _G, k, optimize=True)
        w = np.zeros((128, 16 * 3 * 64), np.float32)
        for xb in range(4):
            for xc in range(4):
                for dai in range(3):
                    cb = (((xb * 4 + xc) * 3) + dai) * 64
                    for s in range(4):
                        for t in range(2):
                            if 0 <= s - t <= 2:
                                w[PHYS[s]:PHYS[s] + 32,
                                  cb + t * 32:cb + t * 32 + 32] = \
                                    kh[:, :, dai, xb, xc, s - t].T
        return w

    # L5 winograd: W5[32s+i, ((xb*4+xc)*3 + dai)*4 + o*2 + t]
    kh5 = np.einsum("up,vq,oiapqd->oiauvd", _G, _G, k5, optimize=True)
    w5 = np.zeros((128, 16 * 3 * 4), np.float32)
    for xb in range(4):
        for xc in range(4):
            for dai in range(3):
                cb = (((xb * 4 + xc) * 3) + dai) * 4
                for s in range(4):
                    for t in range(2):
                        if 0 <= s - t <= 2:
                            for o in range(2):
                                w5[PHYS[s]:PHYS[s] + 32, cb + o * 2 + t] = \
                                    kh5[o, :, dai, xb, xc, s - t]

    return ([w0.astype(np.float16)] +
            [pack_mid_wino(k).astype(np.float16) for k in (k1, k2, k3, k4)] +
            [w5.astype(np.float16)])


def _fwd_transform(nc, xw4, tbf, xtv, slot, a):
    """Forward Winograd transform of X'' slice a -> ring slot.

    xw4: X'' view [128, a, b16, c16, j8]; tbf: [128, 4096] staging;
    xtv: ring view [128, slot, xb, xc, bt, ct, j]."""
    t4 = tbf.rearrange("p (xb bt c j) -> p xb bt c j", xb=4, bt=8, c=16, j=8)
    x3 = xw4[:, a]
    # b-stage: B^T rows over b-windows 2bt-1..2bt+2
    # r0 = x[2bt-1] - x[2bt+1]   (bt=0 edge: -x[1])
    nc.vector.tensor_tensor(t4[:, 0, 1:8], x3[:, 1:14:2], x3[:, 3:16:2], op=SUB)
    nc.scalar.activation(t4[:, 0, 0:1], x3[:, 1:2],
                         mybir.ActivationFunctionType.Copy, scale=-1.0)
    # r1 = x[2bt] + x[2bt+1]
    nc.vector.tensor_tensor(t4[:, 1], x3[:, 0:16:2], x3[:, 1:16:2], op=ADD)
    # r2 = x[2bt+1] - x[2bt]
    nc.vector.tensor_tensor(t4[:, 2], x3[:, 1:16:2], x3[:, 0:16:2], op=SUB)
    # r3 = x[2bt] - x[2bt+2]   (bt=7 edge: x[14])
    nc.vector.tensor_tensor(t4[:, 3, 0:7], x3[:, 0:14:2], x3[:, 2:16:2], op=SUB)
    nc.scalar.copy(t4[:, 3, 7:8], x3[:, 14:15])
    # c-stage into the ring slot
    xs = xtv[:, slot]
    nc.vector.tensor_tensor(xs[:, :, 0, :, 1:8],
                            t4[:, :, :, 1:14:2], t4[:, :, :, 3:16:2], op=SUB)
    nc.scalar.activation(xs[:, :, 0, :, 0:1], t4[:, :, :, 1:2],
                         mybir.ActivationFunctionType.Copy, scale=-1.0)
    nc.gpsimd.tensor_tensor(xs[:, :, 1],
                            t4[:, :, :, 0:16:2], t4[:, :, :, 1:16:2], op=ADD)
    nc.vector.tensor_tensor(xs[:, :, 2],
                            t4[:, :, :, 1:16:2], t4[:, :, :, 0:16:2], op=SUB)
    nc.vector.tensor_tensor(xs[:, :, 3, :, 0:7],
                            t4[:, :, :, 0:14:2], t4[:, :, :, 2:16:2], op=SUB)
    nc.scalar.copy(xs[:, :, 3, :, 7:8], t4[:, :, :, 14:15])


def _inverse_c(nc, t3, tch, u):
    """c-stage of the inverse Winograd transform: T_b -> U spatial.

    t3: [128, rb2, k2, 512] where partition half h holds xc = 2k + h.
    tch: [64, 2048] staging — partition-remapped copy of t3's upper half
    (TensorTensor requires equal base partitions for two SBUF inputs, so
    the odd-xc data is first relocated to base 0 with a 1-input copy).
    u: [64, 2048] spatial sum, layout (bt, rb, ct, rc, j)."""
    # xc0=(k0,h0) xc1=(k0,h1) xc2=(k1,h0) xc3=(k1,h1)
    nc.vector.tensor_copy(tch[:], t3[64:128])
    tc3 = tch.rearrange("p (rb k f) -> p rb k f", rb=2, k=2, f=512)
    u4 = u.rearrange("p (b c j) -> p b c j", b=16, c=16, j=8)
    for rb in range(2):
        lo0, lo1 = t3[0:64, rb, 0], t3[0:64, rb, 1]
        hi0, hi1 = tc3[:, rb, 0], tc3[:, rb, 1]
        # rc=0: xc0 + xc1 + xc2
        dst = u4[:, rb:16:2, 0:16:2, :]
        nc.vector.tensor_tensor(dst, lo0, hi0, op=ADD)
        nc.vector.tensor_tensor(dst, dst, lo1, op=ADD)
        # rc=1: xc1 - xc2 - xc3
        dst = u4[:, rb:16:2, 1:16:2, :]
        nc.vector.tensor_tensor(dst, hi0, lo1, op=SUB)
        nc.vector.tensor_tensor(dst, dst, hi1, op=SUB)


@with_exitstack
def _conv_kernel(ctx, tc, la, slopes):
    """Emit the full 6-layer conv program. slopes: python floats len 5."""
    nc = tc.nc
    na4 = la // 4
    xcols = la * LB * 16 * J          # X'' free size (a, b, c, j)
    pcols = LB * CP * DP

    xin = nc.dram_tensor("xin", [2 * la, 4096], F32, kind="ExternalInput")
    w0_d = nc.dram_tensor("w0", [128, len(G_L0) * na4 * 128],
                          F16, kind="ExternalInput")
    wmid_d = [nc.dram_tensor(f"w{l}", [128, 16 * 3 * 64], F16,
                             kind="ExternalInput") for l in (1, 2, 3, 4)]
    w5_d = nc.dram_tensor("w5", [128, 16 * 3 * 4], F16, kind="ExternalInput")
    out_d = nc.dram_tensor("out_dec", [4, la * 2048], F16,
                           kind="ExternalOutput")

    const = ctx.enter_context(tc.tile_pool(name="const", bufs=1))
    pp = ctx.enter_context(tc.tile_pool(name="ps", bufs=4, space="PSUM"))

    # ---- persistent tiles ----
    w0t = const.tile([128, w0_d.shape[1]], F16, tag="w0")
    nc.sync.dma_start(w0t[:], w0_d[:])
    wmt = []
    for wd in wmid_d:
        t = const.tile([128, 16 * 3 * 64], F16, tag=wd.name)
        nc.sync.dma_start(t[:], wd[:])
        wmt.append(t)
    w5t = const.tile([128, 16 * 3 * 4], F16, tag="w5")
    nc.sync.dma_start(w5t[:], w5_d[:])

    xw = const.tile([128, xcols], F16, tag="xw")          # spatial X''
    xt = const.tile([128, 4 * 8192], F16, tag="xt")       # winograd ring
    xw4 = xw.rearrange("p (a b c j) -> p a b c j", a=la, b=LB, c=16, j=J)
    xtv = xt.rearrange("p (s xb xc bt ct j) -> p s xb xc bt ct j",
                       s=4, xb=4, xc=4, bt=8, ct=8, j=8)

    # zero the never-written d-edge slots once (s=0 j=0; s=3 j=7)
    nc.vector.memset(xw4[64:96, :, :, :, 0:1], 0.0)
    nc.gpsimd.memset(xw4[96:128, :, :, :, 7:8], 0.0)

    # ================= layer 0 =================
    with tc.tile_pool(name="l0", bufs=1) as l0p:
        xsb = l0p.tile([2 * la, 4096], F32, tag="xsb")
        nc.sync.dma_start(xsb[:], xin[:])
        xpad = l0p.tile([128, pcols], F16, tag="xpad")
        nc.vector.memset(xpad[:], 0.0)
        xp4 = xpad.rearrange("p (b c d) -> p b c d", b=LB, c=CP, d=DP)
        xs4 = xsb.rearrange("p (b c d) -> p b c d", b=LB, c=16, d=16)
        # block rg holds x shifted in d by dd = rg - 1 (zero-padded edges)
        for rg in range(3):
            dst = xp4[32 * rg:32 * rg + 2 * la, :, 1:17, 2 - rg:18 - rg]
            if rg == 1:
                nc.gpsimd.tensor_copy(dst, xs4[:])
            else:
                nc.scalar.copy(dst, xs4[:])

        SMAP = {0: (1, 7, 1), 1: (0, 8, 0), 2: (0, 8, 1), 3: (0, 7, 2)}
        a_slope = slopes[0]
        for a0b in range(na4):
            if a0b == 1:
                # X'' slices 0..3 are complete: prime the L1 ring now so the
                # transforms overlap the rest of L0 on the PE
                for a2 in (0, 1, 2):
                    tbf0 = l0p.tile([128, 4096], F16, tag="tbf0", bufs=2)
                    _fwd_transform(nc, xw4, tbf0, xtv, a2 % 4, a2)
            for bc in range(8):          # b-pair chunks
                b0 = bc * 2
                ps = pp.tile([128, 512], F32, tag="ps")
                p4 = ps.rearrange("p (b c d) -> p b c d", b=2, c=16, d=16)
                for gi, (db, dc) in enumerate(G_L0):
                    blo = max(b0, -db)
                    bhi = min(b0 + 2, 16 - db)
                    cb = (gi * na4 + a0b) * 128
                    rhs = xp4[0:96, blo + db:bhi + db, dc + 1:dc + 17, 1:17]
                    out = p4[:, blo - b0:bhi - b0, :, :]
                    nc.tensor.matmul(out, w0t[0:96, cb:cb + 128], rhs,
                                     start=(gi == 0),
                                     stop=(gi == len(G_L0) - 1))
                sg = l0p.tile([128, 512], F16, tag="l0st", bufs=4)
                if USE_LRELU:
                    nc.scalar.activation(sg[:], ps[:],
                                         mybir.ActivationFunctionType.Lrelu,
                                         alpha=a_slope)
                else:
                    nc.scalar.activation(sg[:], ps[:],
                                         mybir.ActivationFunctionType.Relu,
                                         scale=1.0 - a_slope)
                    nc.vector.scalar_tensor_tensor(
                        sg[:], ps[:], a_slope, sg[:],
                        op0=mybir.AluOpType.mult, op1=mybir.AluOpType.add)
                sg4 = sg.rearrange("p (b c d) -> p b c d", b=2, c=16, d=16)
                for aj in range(4):
                    a = a0b * 4 + aj
                    for s in range(4):
                        jlo, jcnt, dlo = SMAP[s]
                        dst = xw4[PHYS[s]:PHYS[s] + 32, a, b0:b0 + 2, :,
                                  jlo:jlo + jcnt]
                        src = sg4[32 * aj:32 * aj + 32, :, :,
                                  dlo:dlo + 2 * jcnt - 1:2]
                        if s in (0, 2):
                            nc.scalar.copy(dst, src)
                        else:
                            nc.vector.tensor_copy(dst, src)

    stg = ctx.enter_context(tc.tile_pool(name="stg", bufs=2))

    def _transform(a2):
        tbf = stg.tile([128, 4096], F16, tag="tbf")
        _fwd_transform(nc, xw4, tbf, xtv, a2 % 4, a2)

    # ================= layers 1..5 (winograd b,c) =================
    for l in range(1, 6):
        is_l5 = (l == 5)
        wt = w5t if is_l5 else wmt[l - 1]
        a_slope = slopes[l] if not is_l5 else 0.0

        def _inverse_chain(pa, s3, t3, u, is_l5=is_l5, a_slope=a_slope):
            """Deferred b-inverse/c-inverse/PReLU/scatter for slice pa."""
            nc.vector.tensor_tensor(t3[:, 0], s3[:, 0:2], s3[:, 2:4], op=ADD)
            nc.vector.tensor_tensor(t3[:, 0], t3[:, 0], s3[:, 4:6], op=ADD)
            nc.vector.tensor_tensor(t3[:, 1], s3[:, 2:4], s3[:, 4:6], op=SUB)
            nc.vector.tensor_tensor(t3[:, 1], t3[:, 1], s3[:, 6:8], op=SUB)
            tch = stg.tile([64, 2048], F16, tag="tch", bufs=1)
            _inverse_c(nc, t3, tch, u)
            if is_l5:
                nc.sync.dma_start(out_d[:, pa * 2048:(pa + 1) * 2048],
                                  u[0:4, :])
                return
            # PReLU into the direct d-slots s'=1,2
            dst = xw4[0:64, pa]
            if USE_LRELU:
                nc.scalar.activation(dst, u[:],
                                     mybir.ActivationFunctionType.Lrelu,
                                     alpha=a_slope)
            else:
                nc.scalar.activation(dst, u[:],
                                     mybir.ActivationFunctionType.Relu,
                                     scale=1.0 - a_slope)
                nc.vector.scalar_tensor_tensor(
                    dst, u[:], a_slope, dst,
                    op0=mybir.AluOpType.mult, op1=mybir.AluOpType.add)
            # j-shift copies: s'=3 <- s'=1 (j+1); s'=0 <- s'=2 (j-1)
            nc.gpsimd.tensor_copy(xw4[96:128, pa, :, :, 0:7],
                                  xw4[0:32, pa, :, :, 1:8])
            nc.gpsimd.tensor_copy(xw4[64:96, pa, :, :, 1:8],
                                  xw4[32:64, pa, :, :, 0:7])

        pending = None
        for a in range(la):
            # ---- forward-transform slice a+2 (slices 0..2 primed) ----
            if 1 <= a and a + 2 < la:
                _transform(a + 2)

            # ---- matmuls: accumulate 3 da taps per xi (dai-outer so the
            # oldest ring slot is released early) ----
            dais = [d for d in (0, 1, 2) if 0 <= a + d - 1 < la]
            sev = stg.tile([128, 4096], F16, tag="sev")
            s3 = sev.rearrange("p (q f) -> p q f", q=8, f=512)
            tbi = stg.tile([128, 2048], F16, tag="tbi", bufs=1)
            t3 = tbi.rearrange("p (rb k f) -> p rb k f", rb=2, k=2, f=512)
            u = stg.tile([64, 2048], F16, tag="u", bufs=1)
            prev = None
            if is_l5:
                # 8 xi per 2-bank tile: xi = tau*8 + k*4 + q, rows 32q, bank k
                pst = [pp.tile([128, 1024], F32, tag="ps", name=f"p5{t_}",
                               bufs=4) for t_ in range(2)]
                for idx, dai in enumerate(dais):
                    slot = (a + dai - 1) % 4
                    for xi in range(16):
                        tau, r = divmod(xi, 8)
                        k, q = divmod(r, 4)
                        rhs = xt[:, slot * 8192 + xi * 512:
                                 slot * 8192 + xi * 512 + 512]
                        out = pst[tau][32 * q:32 * q + 4,
                                       512 * k:512 * k + 512]
                        cb = (xi * 3 + dai) * 4
                        mm = nc.tensor.matmul(out, wt[:, cb:cb + 4], rhs,
                                              start=(idx == 0),
                                              stop=(idx == len(dais) - 1),
                                              tile_position=(0, 32 * q))
                        if prev is not None:
                            tile.add_dep_helper(mm.ins, prev.ins,
                                                sync=False, reason="order")
                        prev = mm
                for xi in range(16):
                    tau, r = divmod(xi, 8)
                    k, qq = divmod(r, 4)
                    h, q = (xi % 4) % 2, 2 * (xi // 4) + (xi % 4) // 2
                    src = pst[tau][32 * qq:32 * qq + 4, 512 * k:512 * k + 512]
                    nc.scalar.copy(s3[64 * h:64 * h + 4, q], src)
            else:
                # two 2-bank psum tiles per xb pair: halves hold (k = xc//2)
                pst = [pp.tile([128, 1024], F32, tag="ps", name=f"pm{q}",
                               bufs=4) for q in range(4)]
                for idx, dai in enumerate(dais):
                    slot = (a + dai - 1) % 4
                    for xi in range(16):
                        xb, xc = divmod(xi, 4)
                        h, k = xc % 2, xc // 2
                        rhs = xt[:, slot * 8192 + xi * 512:
                                 slot * 8192 + xi * 512 + 512]
                        out = pst[xb][64 * h:64 * h + 64,
                                      512 * k:512 * k + 512]
                        cb = (xi * 3 + dai) * 64
                        mm = nc.tensor.matmul(out, wt[:, cb:cb + 64], rhs,
                                              start=(idx == 0),
                                              stop=(idx == len(dais) - 1),
                                              tile_position=(0, 64 * h))
                        if prev is not None:
                            tile.add_dep_helper(mm.ins, prev.ins,
                                                sync=False, reason="order")
                        prev = mm
                # S free layout q = 2*xb + k -> one 2-bank evict per xb
                for xb in range(4):
                    nc.scalar.copy(sev[:, xb * 1024:(xb + 1) * 1024],
                                   pst[xb][:])

            # ---- deferred inverse chain of the previous slice ----
            if pending is not None:
                _inverse_chain(*pending)
            pending = (a, s3, t3, u)

            # ---- hoist the next layer's ring primes into this one ----
            if a >= la - 3 and l < 5:
                _transform(a - (la - 3))

        _inverse_chain(*pending)


_CACHE = {}
LAST_RESULT = None


def _build(la, slopes):
    key = (la, tuple(slopes))
    if key not in _CACHE:
        nc = bacc.Bacc("TRN2")
        with tile.TileContext(nc) as tc:
            _conv_kernel(tc, la, slopes)
        nc.compile()
        _CACHE[key] = nc
    return _CACHE[key]


def kernel(x, k0, k1, k2, k3, k4, k5, slopes):
    x = np.asarray(x, np.float32)
    n, _, la = x.shape[:3]
    slopes_f = [float(s) for s in np.asarray(slopes, np.float32)]
    ws = _pack_weights((k0, k1, k2, k3, k4, k5), la)
    nc = _build(la, slopes_f)

    in_maps = []
    for i in range(n):
        m = {"xin": np.ascontiguousarray(x[i].reshape(2 * la, 4096)),
             "w0": ws[0], "w5": ws[5]}
        for l in (1, 2, 3, 4):
            m[f"w{l}"] = ws[l]
        in_maps.append(m)

    res = bass_utils.run_bass_kernel_spmd(nc, in_maps,
                                          core_ids=list(range(n)))
    global LAST_RESULT
    LAST_RESULT = res
    outs = []
    for i in range(n):
        od = np.asarray(res.results[i]["out_dec"], np.float32)
        od = od.reshape(2, 2, la, 16, 16, 8)
        # [o, t, a, b, c, j] -> [o, a, b, c, j, t] -> d = 2j + t
        o = np.transpose(od, (0, 2, 3, 4, 5, 1)).reshape(2, la, 16, 16, 16)
        outs.append(o)
    return np.stack(outs).astype(np.float32)
